# revision 5
# baseline (speedup 1.0000x reference)
"""Bahdanau attention on 8 TRN2 NeuronCores, data-parallel over batch.

Math (per batch b):
    query  = decoder_hidden @ W_q.T                    # [H]
    keys   = encoder_outputs[b] @ W_k.T                # [S, H]
    energy = tanh(query + keys)                        # [S, H]
    scores = energy @ v                                # [S]
    attention_weights = softmax(scores)                # [S]
    context = attention_weights @ encoder_outputs[b]   # [H]

Device layout ("T domain"): everything keeps the hidden dim on SBUF
partitions. The host pre-transposes encoder_outputs to [B, H, S] so each
core streams encT tiles [128h x 512s] at full DMA speed. Keys matmuls run
with W_k chunks stationary (f32r = full-rate 4-byte matmul), tanh+query-add
is fused on the scalar engine (query as per-partition bias), scores use a
PE matvec against v (contraction over partitions), softmax runs
denominator-free via Exp with fused accumulation, and the context sum
(contraction over s, the free axis) runs on the vector engine as fused
multiply+reduce against a PE-broadcast of the weights.

Each core handles 8 batches; no collectives.
"""

import numpy as np
from contextlib import ExitStack

B, S, H = 64, 2048, 512
N_CORES = 8
B_LOC = B // N_CORES  # 8 batches per core
P = 128
HC = H // P           # 4 h-chunks (partition blocks of the hidden dim)
GC = H // P           # 4 g-chunks (output blocks of the keys matmul)
S_TILE = 512
SC = S // S_TILE      # 4 s-chunks

_CACHE = {}


def build_bass(n_iters=1):
    """Build + compile the per-core program. n_iters>1 repeats the batch loop
    (same outputs) for in-NEFF timing."""
    if n_iters in _CACHE:
        return _CACHE[n_iters]

    import concourse.bass as bass
    import concourse.tile as tile
    from concourse import bacc, mybir

    f32 = mybir.dt.float32
    f32r = mybir.dt.float32r
    AF = mybir.ActivationFunctionType
    ALU = mybir.AluOpType

    nc = bacc.Bacc("TRN2", target_bir_lowering=False, debug=False,
                   num_devices=N_CORES)

    encT = nc.dram_tensor("encT", [B_LOC, H, S], f32r, kind="ExternalInput").ap()
    dhT = nc.dram_tensor("dhT", [H, B_LOC], f32, kind="ExternalInput").ap()
    wkT = nc.dram_tensor("wkT", [H, H], f32r, kind="ExternalInput").ap()
    wqT = nc.dram_tensor("wqT", [H, H], f32, kind="ExternalInput").ap()
    v_in = nc.dram_tensor("v", [H], f32r, kind="ExternalInput").ap()
    w_out = nc.dram_tensor("w_out", [B_LOC, S], f32, kind="ExternalOutput").ap()
    ctxT_out = nc.dram_tensor("ctxT_out", [P, HC * B_LOC], f32,
                              kind="ExternalOutput").ap()

    with tile.TileContext(nc) as tc, ExitStack() as ctx:
        consts = ctx.enter_context(tc.tile_pool(name="consts", bufs=1))
        encp = ctx.enter_context(tc.tile_pool(name="encp", bufs=2))
        enrg = ctx.enter_context(tc.tile_pool(name="energy", bufs=2))
        scrp = ctx.enter_context(tc.tile_pool(name="scratch", bufs=1))
        smal = ctx.enter_context(tc.tile_pool(name="small", bufs=2))
        # PSUM: pe0..pe3 single-buffered (4 banks) + ps/pw double (4 banks)
        psum1 = ctx.enter_context(tc.tile_pool(name="psum1", bufs=1, space="PSUM"))
        psum2 = ctx.enter_context(tc.tile_pool(name="psum2", bufs=2, space="PSUM"))

        # ---- constants ----
        wk_sb, wq_sb, dh_sb = [], [], []
        for h in range(HC):
            wk = consts.tile([P, H], f32r, tag=f"wk{h}")
            nc.sync.dma_start(wk[:], wkT[h * P:(h + 1) * P, :])
            wk_sb.append(wk)
            wq = consts.tile([P, H], f32, tag=f"wq{h}")
            nc.sync.dma_start(wq[:], wqT[h * P:(h + 1) * P, :])
            wq_sb.append(wq)
            dh = consts.tile([P, B_LOC], f32, tag=f"dh{h}")
            nc.sync.dma_start(dh[:], dhT[h * P:(h + 1) * P, :])
            dh_sb.append(dh)
        v_sb = consts.tile([P, GC], f32r, tag="v")
        nc.sync.dma_start(v_sb[:], v_in.rearrange("(c p) -> p c", p=P))
        ones_sb = consts.tile([1, P], f32, tag="ones")
        nc.vector.memset(ones_sb[:], 1.0)

        # ---- query: qT[g, b] = sum_h WqT[h, g] * dhT[h, b]  (exact fp32) ----
        qT_sb = []
        for g in range(GC):
            pq = psum2.tile([P, B_LOC], f32, tag="pw")
            for h in range(HC):
                nc.tensor.matmul(out=pq[:], lhsT=wq_sb[h][:, g * P:(g + 1) * P],
                                 rhs=dh_sb[h][:], start=(h == 0), stop=(h == HC - 1))
            q = consts.tile([P, B_LOC], f32, tag=f"qT{g}")
            nc.scalar.copy(q[:], pq[:])
            qT_sb.append(q)

        ctx_stage = consts.tile([P, HC * B_LOC], f32, tag="ctxstage")

        for _ in range(n_iters):
            for b in range(B_LOC):
                enc_sb = [encp.tile([P, S], f32r, tag=f"enc{h}", name=f"enc{h}")
                          for h in range(HC)]
                for h in range(HC):
                    nc.sync.dma_start(enc_sb[h][:], encT[b, h * P:(h + 1) * P, :])

                esc = smal.tile([1, S], f32, tag="esc")
                dnp = smal.tile([1, SC], f32, tag="dnp")
                for sc in range(SC):
                    ssl = bass.ds(sc * S_TILE, S_TILE)
                    en_g = []
                    for g in range(GC):
                        pe = psum1.tile([P, S_TILE], f32, tag=f"pe{g}")
                        for h in range(HC):
                            nc.tensor.matmul(
                                out=pe[:],
                                lhsT=wk_sb[h][:, g * P:(g + 1) * P],
                                rhs=enc_sb[h][:, ssl],
                                start=(h == 0), stop=(h == HC - 1))
                        en = enrg.tile([P, S_TILE], f32r, tag=f"en{g}")
                        nc.scalar.activation(en[:], pe[:], AF.Tanh,
                                             bias=qT_sb[g][:, b:b + 1], scale=1.0)
                        en_g.append(en)
                    ps = psum2.tile([1, S_TILE], f32, tag="ps")
                    for g in range(GC):
                        nc.tensor.matmul(out=ps[:],
                                         lhsT=v_sb[:, g:g + 1],
                                         rhs=en_g[g][:],
                                         start=(g == 0), stop=(g == GC - 1))
                    nc.scalar.activation(esc[:, ssl], ps[:], AF.Exp,
                                         accum_out=dnp[:, sc:sc + 1])

                dn = smal.tile([1, 1], f32, tag="dn")
                nc.vector.tensor_reduce(dn[:], dnp[:], axis=mybir.AxisListType.X,
                                        op=ALU.add)
                inv = smal.tile([1, 1], f32, tag="inv")
                nc.vector.reciprocal(inv[:], dn[:])
                wsb = smal.tile([1, S], f32, tag="w")
                nc.scalar.activation(wsb[:], esc[:], AF.Copy, scale=inv[:])
                nc.sync.dma_start(w_out[b:b + 1, :], wsb[:])

                scr_sb = [scrp.tile([P, S], f32, tag=f"scr{h}", name=f"scr{h}")
                          for h in range(HC)]
                for sc in range(SC):
                    ssl = bass.ds(sc * S_TILE, S_TILE)
                    pw = psum2.tile([P, S_TILE], f32, tag="pw")
                    nc.tensor.matmul(out=pw[:], lhsT=ones_sb[:],
                                     rhs=wsb[:, ssl],
                                     start=True, stop=True)
                    for h in range(HC):
                        nc.vector.tensor_mul(
                            scr_sb[h][:, ssl],
                            enc_sb[h][:, ssl].bitcast(f32), pw[:])
                for h in range(HC):
                    col = h * B_LOC + b
                    nc.vector.tensor_reduce(
                        ctx_stage[:, col:col + 1], scr_sb[h][:],
                        axis=mybir.AxisListType.X, op=ALU.add)

        nc.sync.dma_start(ctxT_out[:, :], ctx_stage[:])

    nc.compile()
    _CACHE[n_iters] = nc
    return nc


def _run(in_maps, n_iters=1, **kw):
    from concourse.bass_utils import run_bass_kernel_spmd
    nc = build_bass(n_iters)
    return run_bass_kernel_spmd(nc, in_maps, core_ids=list(range(N_CORES)), **kw)


def make_in_maps(decoder_hidden, encoder_outputs, W_q, W_k, v):
    dh = np.asarray(decoder_hidden, dtype=np.float32)
    enc = np.asarray(encoder_outputs, dtype=np.float32)
    encT = np.ascontiguousarray(enc.transpose(0, 2, 1))  # [B, H, S]
    wkT = np.ascontiguousarray(np.asarray(W_k, dtype=np.float32).T)
    wqT = np.ascontiguousarray(np.asarray(W_q, dtype=np.float32).T)
    v_np = np.ascontiguousarray(np.asarray(v, dtype=np.float32))
    in_maps = []
    for c in range(N_CORES):
        b0 = c * B_LOC
        in_maps.append({
            "encT": np.ascontiguousarray(encT[b0:b0 + B_LOC]),
            "dhT": np.ascontiguousarray(dh[b0:b0 + B_LOC].T),
            "wkT": wkT,
            "wqT": wqT,
            "v": v_np,
        })
    return in_maps


def assemble(results):
    ctx_out = np.empty((B, H), np.float32)
    att = np.empty((B, S), np.float32)
    for c in range(N_CORES):
        r = results[c]
        att[c * B_LOC:(c + 1) * B_LOC] = r["w_out"]
        ctx_out[c * B_LOC:(c + 1) * B_LOC] = (
            r["ctxT_out"].reshape(P, HC, B_LOC).transpose(2, 1, 0)
            .reshape(B_LOC, H))
    return ctx_out, att


def kernel(decoder_hidden, encoder_outputs, W_q, W_k, v):
    in_maps = make_in_maps(decoder_hidden, encoder_outputs, W_q, W_k, v)
    res = _run(in_maps)
    return assemble(res.results)


# revision 8
# speedup vs baseline: 348.9103x; 348.9103x over previous
"""Bahdanau attention on 8 TRN2 NeuronCores, data-parallel over batch.

Math (per batch b):
    query  = decoder_hidden @ W_q.T                    # [H]
    keys   = encoder_outputs[b] @ W_k.T                # [S, H]
    energy = tanh(query + keys)                        # [S, H]
    scores = energy @ v                                # [S]
    attention_weights = softmax(scores)                # [S]
    context = attention_weights @ encoder_outputs[b]   # [H]

Device layout ("T domain"): everything keeps the hidden dim on SBUF
partitions. The host pre-transposes encoder_outputs to [B, H, S] so each
core streams encT tiles [128h x 512s] at full DMA speed. Keys matmuls run
with W_k chunks stationary (f32r = full-rate 4-byte matmul), tanh+query-add
is fused on the scalar engine (query as per-partition bias), scores use a
PE matvec against v (contraction over partitions), softmax runs
denominator-free via Exp with fused accumulation, and the context sum
(contraction over s, the free axis) runs on the vector engine as fused
multiply+reduce against a PE-broadcast of the weights.

Each core handles 8 batches; no collectives.
"""

import numpy as np
from contextlib import ExitStack

B, S, H = 64, 2048, 512
N_CORES = 8
B_LOC = B // N_CORES  # 8 batches per core
P = 128
HC = H // P           # 4 h-chunks (partition blocks of the hidden dim)
GC = H // P           # 4 g-chunks (output blocks of the keys matmul)
S_TILE = 512
SC = S // S_TILE      # 4 s-chunks

_CACHE = {}


def build_bass(n_iters=1):
    """Build + compile the per-core program. n_iters>1 repeats the batch loop
    (same outputs) for in-NEFF timing."""
    if n_iters in _CACHE:
        return _CACHE[n_iters]

    import concourse.bass as bass
    import concourse.tile as tile
    from concourse import bacc, mybir

    f32 = mybir.dt.float32
    f32r = mybir.dt.float32r
    AF = mybir.ActivationFunctionType
    ALU = mybir.AluOpType

    nc = bacc.Bacc("TRN2", target_bir_lowering=False, debug=False,
                   num_devices=N_CORES)

    encT = nc.dram_tensor("encT", [B_LOC, H, S], f32r, kind="ExternalInput").ap()
    dhT = nc.dram_tensor("dhT", [H, B_LOC], f32, kind="ExternalInput").ap()
    wkT = nc.dram_tensor("wkT", [H, H], f32r, kind="ExternalInput").ap()
    wqT = nc.dram_tensor("wqT", [H, H], f32, kind="ExternalInput").ap()
    v_in = nc.dram_tensor("v", [H], f32r, kind="ExternalInput").ap()
    w_out = nc.dram_tensor("w_out", [B_LOC, S], f32, kind="ExternalOutput").ap()
    ctxT_out = nc.dram_tensor("ctxT_out", [P, HC * B_LOC], f32,
                              kind="ExternalOutput").ap()

    with tile.TileContext(nc) as tc, ExitStack() as ctx:
        consts = ctx.enter_context(tc.tile_pool(name="consts", bufs=1))
        encp = ctx.enter_context(tc.tile_pool(name="encp", bufs=2))
        enrg = ctx.enter_context(tc.tile_pool(name="energy", bufs=2))
        scrp = ctx.enter_context(tc.tile_pool(name="scratch", bufs=1))
        smal = ctx.enter_context(tc.tile_pool(name="small", bufs=2))
        # PSUM: pe0..pe3 single-buffered (4 banks) + ps/pw double (4 banks)
        psum1 = ctx.enter_context(tc.tile_pool(name="psum1", bufs=1, space="PSUM"))
        psum2 = ctx.enter_context(tc.tile_pool(name="psum2", bufs=2, space="PSUM"))

        # ---- constants ----
        wk_sb, wq_sb, dh_sb = [], [], []
        for h in range(HC):
            wk = consts.tile([P, H], f32r, tag=f"wk{h}")
            nc.sync.dma_start(wk[:], wkT[h * P:(h + 1) * P, :])
            wk_sb.append(wk)
            wq = consts.tile([P, H], f32, tag=f"wq{h}")
            nc.sync.dma_start(wq[:], wqT[h * P:(h + 1) * P, :])
            wq_sb.append(wq)
            dh = consts.tile([P, B_LOC], f32, tag=f"dh{h}")
            nc.sync.dma_start(dh[:], dhT[h * P:(h + 1) * P, :])
            dh_sb.append(dh)
        v_sb = consts.tile([P, GC], f32r, tag="v")
        nc.sync.dma_start(v_sb[:], v_in.rearrange("(c p) -> p c", p=P))
        ones_sb = consts.tile([1, P], f32, tag="ones")
        nc.vector.memset(ones_sb[:], 1.0)

        # ---- query: qT[g, b] = sum_h WqT[h, g] * dhT[h, b]  (exact fp32) ----
        qT_sb = []
        for g in range(GC):
            pq = psum2.tile([P, B_LOC], f32, tag="pw")
            for h in range(HC):
                nc.tensor.matmul(out=pq[:], lhsT=wq_sb[h][:, g * P:(g + 1) * P],
                                 rhs=dh_sb[h][:], start=(h == 0), stop=(h == HC - 1))
            q = consts.tile([P, B_LOC], f32, tag=f"qT{g}")
            nc.scalar.copy(q[:], pq[:])
            qT_sb.append(q)

        ctx_stage = consts.tile([P, HC * B_LOC], f32, tag="ctxstage")

        for _ in range(n_iters):
            for b in range(B_LOC):
                enc_sb = [encp.tile([P, S], f32r, tag=f"enc{h}", name=f"enc{h}")
                          for h in range(HC)]
                for h in range(HC):
                    nc.sync.dma_start(enc_sb[h][:], encT[b, h * P:(h + 1) * P, :])

                esc = smal.tile([1, S], f32, tag="esc")
                dnp = smal.tile([1, SC], f32, tag="dnp")
                for sc in range(SC):
                    ssl = bass.ds(sc * S_TILE, S_TILE)
                    en_g = []
                    for g in range(GC):
                        pe = psum1.tile([P, S_TILE], f32, tag=f"pe{g}")
                        for h in range(HC):
                            nc.tensor.matmul(
                                out=pe[:],
                                lhsT=wk_sb[h][:, g * P:(g + 1) * P],
                                rhs=enc_sb[h][:, ssl],
                                start=(h == 0), stop=(h == HC - 1))
                        en = enrg.tile([P, S_TILE], f32r, tag=f"en{g}")
                        nc.scalar.activation(en[:], pe[:], AF.Tanh,
                                             bias=qT_sb[g][:, b:b + 1], scale=1.0)
                        en_g.append(en)
                    ps = psum2.tile([1, S_TILE], f32, tag="ps")
                    for g in range(GC):
                        nc.tensor.matmul(out=ps[:],
                                         lhsT=v_sb[:, g:g + 1],
                                         rhs=en_g[g][:],
                                         start=(g == 0), stop=(g == GC - 1))
                    nc.scalar.activation(esc[:, ssl], ps[:], AF.Exp,
                                         accum_out=dnp[:, sc:sc + 1])

                dn = smal.tile([1, 1], f32, tag="dn")
                nc.vector.tensor_reduce(dn[:], dnp[:], axis=mybir.AxisListType.X,
                                        op=ALU.add)
                inv = smal.tile([1, 1], f32, tag="inv")
                nc.vector.reciprocal(inv[:], dn[:])
                wsb = smal.tile([1, S], f32, tag="w")
                nc.scalar.activation(wsb[:], esc[:], AF.Copy, scale=inv[:])
                nc.sync.dma_start(w_out[b:b + 1, :], wsb[:])

                scr_sb = [scrp.tile([P, S], f32, tag=f"scr{h}", name=f"scr{h}")
                          for h in range(HC)]
                for sc in range(SC):
                    ssl = bass.ds(sc * S_TILE, S_TILE)
                    pw = psum2.tile([P, S_TILE], f32, tag="pw")
                    nc.tensor.matmul(out=pw[:], lhsT=ones_sb[:],
                                     rhs=wsb[:, ssl],
                                     start=True, stop=True)
                    for h in range(HC):
                        nc.vector.tensor_mul(
                            scr_sb[h][:, ssl],
                            enc_sb[h][:, ssl].bitcast(f32), pw[:])
                for h in range(HC):
                    col = h * B_LOC + b
                    nc.vector.tensor_reduce(
                        ctx_stage[:, col:col + 1], scr_sb[h][:],
                        axis=mybir.AxisListType.X, op=ALU.add)

        nc.sync.dma_start(ctxT_out[:, :], ctx_stage[:])

    nc.compile()
    _CACHE[n_iters] = nc
    return nc


def _run(in_maps, n_iters=1, **kw):
    from concourse.bass_utils import run_bass_kernel_spmd
    nc = build_bass(n_iters)
    return run_bass_kernel_spmd(nc, in_maps, core_ids=list(range(N_CORES)), **kw)


def make_runner(n_iters, in_maps):
    """Jit the NEFF once and keep inputs device-resident; returns a zero-arg
    callable that executes one dispatch and blocks until done."""
    import jax
    import numpy as _np
    from jax.sharding import Mesh, PartitionSpec, NamedSharding
    from jax.experimental.shard_map import shard_map
    from concourse import bass2jax, mybir as _mybir

    nc = build_bass(n_iters)
    bass2jax.install_neuronx_cc_hook()

    partition_name = (nc.partition_id_tensor.name
                      if nc.partition_id_tensor else None)
    in_names, out_names, out_avals, zero_shapes = [], [], [], []
    for alloc in nc.m.functions[0].allocations:
        if not isinstance(alloc, _mybir.MemoryLocationSet):
            continue
        name = alloc.memorylocations[0].name
        if alloc.kind == "ExternalInput":
            if name != partition_name:
                in_names.append(name)
        elif alloc.kind == "ExternalOutput":
            out_names.append(name)
            shape = tuple(alloc.tensor_shape)
            dtype = _mybir.dt.np(alloc.dtype)
            out_avals.append(jax.core.ShapedArray(shape, dtype))
            zero_shapes.append((shape, dtype))
    n_params = len(in_names)
    all_names = in_names + out_names
    if partition_name is not None:
        all_names = all_names + [partition_name]

    def _body(*args):
        operands = list(args)
        if partition_name is not None:
            operands.append(bass2jax.partition_id_tensor())
        outs = bass2jax._bass_exec_p.bind(
            *operands,
            out_avals=tuple(out_avals),
            in_names=tuple(all_names),
            out_names=tuple(out_names),
            lowering_input_output_aliases=(),
            sim_require_finite=True,
            sim_require_nnan=True,
            nc=nc,
        )
        return tuple(outs)

    devices = jax.devices()[:N_CORES]
    mesh = Mesh(_np.asarray(devices), ("core",))
    spec = NamedSharding(mesh, PartitionSpec("core"))
    n_outs = len(out_names)
    donate = tuple(range(n_params, n_params + n_outs))
    sharded = jax.jit(
        shard_map(_body, mesh=mesh,
                  in_specs=(PartitionSpec("core"),) * (n_params + n_outs),
                  out_specs=(PartitionSpec("core"),) * n_outs,
                  check_rep=False),
        donate_argnums=donate, keep_unused=True)

    concat_in = [
        jax.device_put(
            _np.concatenate([_np.asarray(in_maps[c][k])[None] for c in
                             range(N_CORES)], axis=0).reshape(
                N_CORES * _np.asarray(in_maps[0][k]).shape[0],
                *_np.asarray(in_maps[0][k]).shape[1:]),
            spec)
        for k in in_names
    ]

    def call():
        zeros = [_np.zeros((N_CORES * s[0], *s[1:]), d) for s, d in zero_shapes]
        outs = sharded(*concat_in, *zeros)
        jax.block_until_ready(outs)
        return outs

    return call


def make_in_maps(decoder_hidden, encoder_outputs, W_q, W_k, v):
    dh = np.asarray(decoder_hidden, dtype=np.float32)
    enc = np.asarray(encoder_outputs, dtype=np.float32)
    encT = np.ascontiguousarray(enc.transpose(0, 2, 1))  # [B, H, S]
    wkT = np.ascontiguousarray(np.asarray(W_k, dtype=np.float32).T)
    wqT = np.ascontiguousarray(np.asarray(W_q, dtype=np.float32).T)
    v_np = np.ascontiguousarray(np.asarray(v, dtype=np.float32))
    in_maps = []
    for c in range(N_CORES):
        b0 = c * B_LOC
        in_maps.append({
            "encT": np.ascontiguousarray(encT[b0:b0 + B_LOC]),
            "dhT": np.ascontiguousarray(dh[b0:b0 + B_LOC].T),
            "wkT": wkT,
            "wqT": wqT,
            "v": v_np,
        })
    return in_maps


def assemble(results):
    ctx_out = np.empty((B, H), np.float32)
    att = np.empty((B, S), np.float32)
    for c in range(N_CORES):
        r = results[c]
        att[c * B_LOC:(c + 1) * B_LOC] = r["w_out"]
        ctx_out[c * B_LOC:(c + 1) * B_LOC] = (
            r["ctxT_out"].reshape(P, HC, B_LOC).transpose(2, 1, 0)
            .reshape(B_LOC, H))
    return ctx_out, att


def kernel(decoder_hidden, encoder_outputs, W_q, W_k, v):
    in_maps = make_in_maps(decoder_hidden, encoder_outputs, W_q, W_k, v)
    res = _run(in_maps)
    return assemble(res.results)


# revision 11
# speedup vs baseline: 2373.4417x; 6.8024x over previous
"""Bahdanau attention on 8 TRN2 NeuronCores, data-parallel over batch.

Math (per batch b):
    query  = decoder_hidden @ W_q.T                    # [H]
    keys   = encoder_outputs[b] @ W_k.T                # [S, H]
    energy = tanh(query + keys)                        # [S, H]
    scores = energy @ v                                # [S]
    attention_weights = softmax(scores)                # [S]
    context = attention_weights @ encoder_outputs[b]   # [H]

Device layout ("T domain"): everything keeps the hidden dim on SBUF
partitions. The host pre-transposes encoder_outputs to [B, H, S] so each
core streams encT tiles [128h x 512s] at full DMA speed. Keys matmuls run
with W_k chunks stationary (f32r = full-rate 4-byte matmul), tanh+query-add
is fused on the scalar engine (query as per-partition bias), scores use a
PE matvec against v (contraction over partitions), softmax runs
denominator-free via Exp with fused accumulation, and the context sum
(contraction over s, the free axis) runs on the vector engine as fused
multiply+reduce against a PE-broadcast of the weights.

Each core handles 8 batches; no collectives.
"""

import numpy as np
from contextlib import ExitStack

B, S, H = 64, 2048, 512
N_CORES = 8
B_LOC = B // N_CORES  # 8 batches per core
P = 128
HC = H // P           # 4 h-chunks (partition blocks of the hidden dim)
GC = H // P           # 4 g-chunks (output blocks of the keys matmul)
S_TILE = 512
SC = S // S_TILE      # 4 s-chunks

_CACHE = {}


def build_bass(n_iters=1):
    """Build + compile the per-core program. n_iters>1 repeats the batch loop
    (same outputs) for in-NEFF timing."""
    if n_iters in _CACHE:
        return _CACHE[n_iters]

    import concourse.bass as bass
    import concourse.tile as tile
    from concourse import bacc, mybir

    f32 = mybir.dt.float32
    f32r = mybir.dt.float32r
    AF = mybir.ActivationFunctionType
    ALU = mybir.AluOpType

    nc = bacc.Bacc("TRN2", target_bir_lowering=False, debug=False,
                   num_devices=N_CORES)

    encT = nc.dram_tensor("encT", [B_LOC, H, S], f32r, kind="ExternalInput").ap()
    dhT = nc.dram_tensor("dhT", [H, B_LOC], f32, kind="ExternalInput").ap()
    wkT = nc.dram_tensor("wkT", [H, H], f32r, kind="ExternalInput").ap()
    wqT = nc.dram_tensor("wqT", [H, H], f32, kind="ExternalInput").ap()
    v_in = nc.dram_tensor("v", [H], f32r, kind="ExternalInput").ap()
    w_out = nc.dram_tensor("w_out", [B_LOC, S], f32r, kind="ExternalOutput").ap()
    ctxT_out = nc.dram_tensor("ctxT_out", [P, HC * B_LOC], f32,
                              kind="ExternalOutput").ap()

    with tile.TileContext(nc) as tc, ExitStack() as ctx:
        consts = ctx.enter_context(tc.tile_pool(name="consts", bufs=1))
        encp = ctx.enter_context(tc.tile_pool(name="encp", bufs=2))
        enrg = ctx.enter_context(tc.tile_pool(name="energy", bufs=2))
        scrp = ctx.enter_context(tc.tile_pool(name="scratch", bufs=2))
        smal = ctx.enter_context(tc.tile_pool(name="small", bufs=2))
        # PSUM: pe0..pe3 single-buffered (4 banks) + ps/pw double (4 banks)
        psum1 = ctx.enter_context(tc.tile_pool(name="psum1", bufs=1, space="PSUM"))
        psum2 = ctx.enter_context(tc.tile_pool(name="psum2", bufs=2, space="PSUM"))

        # ---- constants ----
        wk_sb, wq_sb, dh_sb = [], [], []
        for h in range(HC):
            wk = consts.tile([P, H], f32r, tag=f"wk{h}")
            nc.sync.dma_start(wk[:], wkT[h * P:(h + 1) * P, :])
            wk_sb.append(wk)
            wq = consts.tile([P, H], f32, tag=f"wq{h}")
            nc.sync.dma_start(wq[:], wqT[h * P:(h + 1) * P, :])
            wq_sb.append(wq)
            dh = consts.tile([P, B_LOC], f32, tag=f"dh{h}")
            nc.sync.dma_start(dh[:], dhT[h * P:(h + 1) * P, :])
            dh_sb.append(dh)
        v_sb = consts.tile([P, GC], f32r, tag="v")
        nc.sync.dma_start(v_sb[:], v_in.rearrange("(c p) -> p c", p=P))
        ones_f32 = consts.tile([1, P], f32, tag="ones_f32")
        nc.vector.memset(ones_f32[:], 1.0)
        ones_sb = consts.tile([1, P], f32r, tag="ones")
        nc.scalar.copy(ones_sb[:], ones_f32[:])

        # ---- query: qT[g, b] = sum_h WqT[h, g] * dhT[h, b]  (exact fp32) ----
        qT_sb = []
        for g in range(GC):
            pq = psum2.tile([P, B_LOC], f32, tag="pw")
            for h in range(HC):
                nc.tensor.matmul(out=pq[:], lhsT=wq_sb[h][:, g * P:(g + 1) * P],
                                 rhs=dh_sb[h][:], start=(h == 0), stop=(h == HC - 1))
            q = consts.tile([P, B_LOC], f32, tag=f"qT{g}")
            nc.scalar.copy(q[:], pq[:])
            qT_sb.append(q)

        ctx_stage = consts.tile([P, HC * B_LOC], f32, tag="ctxstage")

        for _ in range(n_iters):
            for b in range(B_LOC):
                enc_sb = [encp.tile([P, S], f32r, tag=f"enc{h}", name=f"enc{h}")
                          for h in range(HC)]
                for h in range(HC):
                    nc.sync.dma_start(enc_sb[h][:], encT[b, h * P:(h + 1) * P, :])

                esc = smal.tile([1, S], f32, tag="esc")
                dnp = smal.tile([1, SC], f32, tag="dnp")
                for sc in range(SC):
                    ssl = bass.ds(sc * S_TILE, S_TILE)
                    en_g = []
                    for g in range(GC):
                        pe = psum1.tile([P, S_TILE], f32, tag=f"pe{g}")
                        for h in range(HC):
                            nc.tensor.matmul(
                                out=pe[:],
                                lhsT=wk_sb[h][:, g * P:(g + 1) * P],
                                rhs=enc_sb[h][:, ssl],
                                start=(h == 0), stop=(h == HC - 1))
                        en = enrg.tile([P, S_TILE], f32r, tag=f"en{g}")
                        nc.scalar.activation(en[:], pe[:], AF.Tanh,
                                             bias=qT_sb[g][:, b:b + 1], scale=1.0)
                        en_g.append(en)
                    ps = psum2.tile([1, S_TILE], f32, tag="ps")
                    for g in range(GC):
                        nc.tensor.matmul(out=ps[:],
                                         lhsT=v_sb[:, g:g + 1],
                                         rhs=en_g[g][:],
                                         start=(g == 0), stop=(g == GC - 1))
                    nc.scalar.activation(esc[:, ssl], ps[:], AF.Exp,
                                         accum_out=dnp[:, sc:sc + 1])

                dn = smal.tile([1, 1], f32, tag="dn")
                nc.vector.tensor_reduce(dn[:], dnp[:], axis=mybir.AxisListType.X,
                                        op=ALU.add)
                inv = smal.tile([1, 1], f32, tag="inv")
                nc.vector.reciprocal(inv[:], dn[:])
                wsb = smal.tile([1, S], f32r, tag="w")
                nc.scalar.activation(wsb[:], esc[:], AF.Copy, scale=inv[:])
                nc.sync.dma_start(w_out[b:b + 1, :], wsb[:])

                acc_sb = [smal.tile([P, SC], f32, tag=f"acc{h}", name=f"acc{h}")
                          for h in range(HC)]
                for sc in range(SC):
                    ssl = bass.ds(sc * S_TILE, S_TILE)
                    pw = psum2.tile([P, S_TILE], f32, tag="pw")
                    nc.tensor.matmul(out=pw[:], lhsT=ones_sb[:],
                                     rhs=wsb[:, ssl],
                                     start=True, stop=True)
                    for h in range(HC):
                        scr = scrp.tile([P, S_TILE], f32, tag=f"scr{h}",
                                        name=f"scr{h}")
                        nc.vector.scalar_tensor_tensor(
                            out=scr[:], in0=enc_sb[h][:, ssl].bitcast(f32),
                            scalar=0.0, in1=pw[:],
                            op0=ALU.bypass, op1=ALU.mult,
                            accum_out=acc_sb[h][:, sc:sc + 1])
                for h in range(HC):
                    col = h * B_LOC + b
                    nc.vector.tensor_reduce(
                        ctx_stage[:, col:col + 1], acc_sb[h][:],
                        axis=mybir.AxisListType.X, op=ALU.add)

        nc.sync.dma_start(ctxT_out[:, :], ctx_stage[:])

    nc.compile()
    _CACHE[n_iters] = nc
    return nc


def _run(in_maps, n_iters=1, **kw):
    from concourse.bass_utils import run_bass_kernel_spmd
    nc = build_bass(n_iters)
    return run_bass_kernel_spmd(nc, in_maps, core_ids=list(range(N_CORES)), **kw)


def make_runner(n_iters, in_maps):
    """Jit the NEFF once and keep inputs device-resident; returns a zero-arg
    callable that executes one dispatch and blocks until done."""
    import jax
    import numpy as _np
    from jax.sharding import Mesh, PartitionSpec, NamedSharding
    from jax.experimental.shard_map import shard_map
    from concourse import bass2jax, mybir as _mybir

    nc = build_bass(n_iters)
    bass2jax.install_neuronx_cc_hook()

    partition_name = (nc.partition_id_tensor.name
                      if nc.partition_id_tensor else None)
    in_names, out_names, out_avals, zero_shapes = [], [], [], []
    for alloc in nc.m.functions[0].allocations:
        if not isinstance(alloc, _mybir.MemoryLocationSet):
            continue
        name = alloc.memorylocations[0].name
        if alloc.kind == "ExternalInput":
            if name != partition_name:
                in_names.append(name)
        elif alloc.kind == "ExternalOutput":
            out_names.append(name)
            shape = tuple(alloc.tensor_shape)
            dtype = _mybir.dt.np(alloc.dtype)
            out_avals.append(jax.core.ShapedArray(shape, dtype))
            zero_shapes.append((shape, dtype))
    n_params = len(in_names)
    all_names = in_names + out_names
    if partition_name is not None:
        all_names = all_names + [partition_name]

    def _body(*args):
        operands = list(args)
        if partition_name is not None:
            operands.append(bass2jax.partition_id_tensor())
        outs = bass2jax._bass_exec_p.bind(
            *operands,
            out_avals=tuple(out_avals),
            in_names=tuple(all_names),
            out_names=tuple(out_names),
            lowering_input_output_aliases=(),
            sim_require_finite=True,
            sim_require_nnan=True,
            nc=nc,
        )
        return tuple(outs)

    devices = jax.devices()[:N_CORES]
    mesh = Mesh(_np.asarray(devices), ("core",))
    spec = NamedSharding(mesh, PartitionSpec("core"))
    n_outs = len(out_names)
    donate = tuple(range(n_params, n_params + n_outs))
    sharded = jax.jit(
        shard_map(_body, mesh=mesh,
                  in_specs=(PartitionSpec("core"),) * (n_params + n_outs),
                  out_specs=(PartitionSpec("core"),) * n_outs,
                  check_rep=False),
        donate_argnums=donate, keep_unused=True)

    concat_in = [
        jax.device_put(
            _np.concatenate([_np.asarray(in_maps[c][k])[None] for c in
                             range(N_CORES)], axis=0).reshape(
                N_CORES * _np.asarray(in_maps[0][k]).shape[0],
                *_np.asarray(in_maps[0][k]).shape[1:]),
            spec)
        for k in in_names
    ]

    def call():
        zeros = [_np.zeros((N_CORES * s[0], *s[1:]), d) for s, d in zero_shapes]
        outs = sharded(*concat_in, *zeros)
        jax.block_until_ready(outs)
        return outs

    return call


def make_in_maps(decoder_hidden, encoder_outputs, W_q, W_k, v):
    dh = np.asarray(decoder_hidden, dtype=np.float32)
    enc = np.asarray(encoder_outputs, dtype=np.float32)
    encT = np.ascontiguousarray(enc.transpose(0, 2, 1))  # [B, H, S]
    wkT = np.ascontiguousarray(np.asarray(W_k, dtype=np.float32).T)
    wqT = np.ascontiguousarray(np.asarray(W_q, dtype=np.float32).T)
    v_np = np.ascontiguousarray(np.asarray(v, dtype=np.float32))
    in_maps = []
    for c in range(N_CORES):
        b0 = c * B_LOC
        in_maps.append({
            "encT": np.ascontiguousarray(encT[b0:b0 + B_LOC]),
            "dhT": np.ascontiguousarray(dh[b0:b0 + B_LOC].T),
            "wkT": wkT,
            "wqT": wqT,
            "v": v_np,
        })
    return in_maps


def assemble(results):
    ctx_out = np.empty((B, H), np.float32)
    att = np.empty((B, S), np.float32)
    for c in range(N_CORES):
        r = results[c]
        att[c * B_LOC:(c + 1) * B_LOC] = r["w_out"]
        ctx_out[c * B_LOC:(c + 1) * B_LOC] = (
            r["ctxT_out"].reshape(P, HC, B_LOC).transpose(2, 1, 0)
            .reshape(B_LOC, H))
    return ctx_out, att


def kernel(decoder_hidden, encoder_outputs, W_q, W_k, v):
    in_maps = make_in_maps(decoder_hidden, encoder_outputs, W_q, W_k, v)
    res = _run(in_maps)
    return assemble(res.results)
